# revision 29
# baseline (speedup 1.0000x reference)
"""Trainium2 Bass kernel for nn_Attention_63273458205325.

Data-parallel over batch: 64 images -> 8 NeuronCores x 8 images.
Host casts x to bf16 (halves HBM traffic); device computes, per image,
the four memory-bound global reductions over x[b] (256x4096):
  - beta row-sums  (per-channel sum over spatial)       [256]
  - mask logits m = w_mask . x  -> exp -> Z and the
    softmax-weighted context sums  sum_s x[c,s]*e[s]    [256]
  - mean over spatial of (max over channels)            scalar
Engine split (per image):
  PE : mask matmul with w_mask replicated into all 128 stationary
       columns -> m arrives pre-broadcast [128, 2048] in PSUM.
  ACT: exp(PSUM)->bf16 SBUF e_b (Z rides accum), plus the two
       beta copy-accum passes.
  DVE: fused multiply-accumulate for ctx (scalar_tensor_tensor),
       channel fold (max), and the post-transpose max tree.
  DMA: x loads on the sync ring; SBUF->SBUF transposes (hardware
       XBAR) split across the sync+scalar rings.
The per-engine queues are software-pipelined one image deep: image
b's slot runs ctx/beta of image b-1 so the in-order DVE/ACT queues
never head-of-line block on the exp->e_b chain. DMA-transpose blocks
its issuing engine until the transfer lands, so even images go on the
(otherwise idle) sync ring and odd images issue from scalar after the
exps, landing the block in dead time.
The tiny [B,8] epilogue head runs on host.
"""

import sys

import numpy as np

sys.path.insert(0, "/opt/trn_rl_repo")

B, C, H, W = 64, 256, 64, 64
S = H * W  # 4096
NCORES = 8
BPC = B // NCORES  # images per core
RATIO, K = 16, 8
PLANES = C // 2
HIDDEN = C // RATIO
TEMP = 30.0
EPS = 1e-5

_CACHE = {}


def _build_nc():
    import concourse.bacc as bacc
    import concourse.mybir as mybir
    from concourse.tile import TileContext

    f32 = mybir.dt.float32
    bf16 = mybir.dt.bfloat16
    AF = mybir.ActivationFunctionType
    ALU = mybir.AluOpType

    nc = bacc.Bacc(None, target_bir_lowering=False)

    x_ext = nc.declare_dram_parameter("x", [BPC, 128, 2 * S], bf16,
                                      isOutput=False)
    wmb0_ext = nc.declare_dram_parameter("wmb0", [128, 128], bf16,
                                         isOutput=False)
    wmb1_ext = nc.declare_dram_parameter("wmb1", [128, 128], bf16,
                                         isOutput=False)
    out_ext = nc.declare_dram_parameter("out", [BPC, 128, 8], f32,
                                        isOutput=True)
    out2_ext = nc.declare_dram_parameter("out2", [BPC, 128, 1024], bf16,
                                         isOutput=True)

    with TileContext(nc) as tc:
        with (
            tc.tile_pool(name="const", bufs=1) as cpool,
            tc.tile_pool(name="xin", bufs=3) as xpool,
            tc.tile_pool(name="work", bufs=3) as wpool,
            tc.tile_pool(name="small", bufs=3) as spool,
            tc.tile_pool(name="stg", bufs=3) as gpool,
            tc.tile_pool(name="psum", bufs=2, space="PSUM") as ppool,
        ):
            scr = cpool.tile([128, S], bf16)
            dummy = cpool.tile([128, 1], bf16)
            wmb0 = cpool.tile([128, 128], bf16)
            wmb1 = cpool.tile([128, 128], bf16)

            xbs, ebs, stages = {}, {}, {}

            def load(b):
                xb = xpool.tile([128, 2 * S], bf16, tag="xb")
                nc.sync.dma_start(out=xb[:], in_=x_ext[b])
                xbs[b] = xb
                stage = gpool.tile([128, 8], f32, tag="stage")
                nc.gpsimd.memset(stage[:], 0.0)
                stages[b] = stage

            pms = {}

            def fold(b):
                xb = xbs[b]
                pm = wpool.tile([128, S], bf16, tag="pm")
                nc.vector.tensor_tensor(out=pm[:], in0=xb[:, 0:S],
                                        in1=xb[:, S:2 * S], op=ALU.max)
                pms[b] = pm

            def transpose(b, eng):
                pm = pms.pop(b)
                pmT = wpool.tile([128, 32, 128], bf16, tag="pmT")
                eng.dma_start(out=pmT[:], in_=pm[:], transpose=True)
                pms[(b, "T")] = pmT

            def tree(b):
                pmT = pms.pop((b, "T"))
                t1 = spool.tile([128, 32, 64], bf16, tag="t1")
                nc.vector.tensor_tensor(out=t1[:], in0=pmT[:, :, 0:64],
                                        in1=pmT[:, :, 64:128], op=ALU.max)
                t2 = spool.tile([128, 32, 32], bf16, tag="t2")
                nc.vector.tensor_tensor(out=t2[:], in0=t1[:, :, 0:32],
                                        in1=t1[:, :, 32:64], op=ALU.max)
                # host finishes the last 5 max levels + the spatial sum
                nc.sync.dma_start(out=out2_ext[b],
                                  in_=t2[:].rearrange("p a b -> p (a b)"))

            def mask_exp(b):
                """PE mask-broadcast matmuls + ACT exp -> e_b, Z."""
                xb, stage = xbs[b], stages[b]
                e_b = wpool.tile([128, S], bf16, tag="eb")
                for g in range(2):
                    mb = ppool.tile([128, 2048], f32, tag="mb")
                    for j in range(4):
                        sl = slice(2048 * g + 512 * j,
                                   2048 * g + 512 * (j + 1))
                        nc.tensor.matmul(mb[:, 512 * j:512 * (j + 1)],
                                         lhsT=wmb0[:], rhs=xb[:, sl],
                                         start=True, stop=False)
                    for j in range(4):
                        sl = slice(S + 2048 * g + 512 * j,
                                   S + 2048 * g + 512 * (j + 1))
                        nc.tensor.matmul(mb[:, 512 * j:512 * (j + 1)],
                                         lhsT=wmb1[:], rhs=xb[:, sl],
                                         start=False, stop=True)
                    nc.scalar.activation(e_b[:, 2048 * g:2048 * (g + 1)],
                                         mb[:], AF.Exp,
                                         accum_out=stage[:, 5 + g:6 + g])
                ebs[b] = e_b

            def beta(b):
                """beta row sums on ACT (copy with accumulator)."""
                xb, stage = xbs[b], stages[b]
                nc.scalar.activation(scr[:], xb[:, 0:S], AF.Copy,
                                     accum_out=stage[:, 0:1])
                nc.scalar.activation(scr[:], xb[:, S:2 * S], AF.Copy,
                                     accum_out=stage[:, 1:2])

            def ctx(b):
                """ctx fused multiply-accumulate on DVE."""
                xb, stage, e_b = xbs[b], stages[b], ebs[b]
                nc.vector.scalar_tensor_tensor(
                    out=dummy[:].broadcast_to([128, S]),
                    in0=xb[:, 0:S], scalar=1.0, in1=e_b[:],
                    op0=ALU.mult, op1=ALU.mult, accum_out=stage[:, 2:3])
                nc.vector.scalar_tensor_tensor(
                    out=dummy[:].broadcast_to([128, S]),
                    in0=xb[:, S:2 * S], scalar=1.0, in1=e_b[:],
                    op0=ALU.mult, op1=ALU.mult, accum_out=stage[:, 3:4])

            def flush(b):
                nc.sync.dma_start(out=out_ext[b], in_=stages[b])
                del xbs[b], ebs[b], stages[b]

            # software pipeline, one image deep
            load(0)
            nc.sync.dma_start(out=wmb0[:], in_=wmb0_ext[:, :])
            nc.sync.dma_start(out=wmb1[:], in_=wmb1_ext[:, :])
            for b in range(BPC):
                if b + 1 < BPC:
                    load(b + 1)
                fold(b)
                if b % 2 == 0:
                    transpose(b, nc.sync)   # blocks the idle sync engine
                if b == BPC - 1:
                    mask_exp(b)     # last slot: unblock the drain's ctx
                if b > 0:
                    beta(b - 1)     # ACT filler while PE streams matmuls
                if b < BPC - 1:
                    mask_exp(b)
                if b % 2 == 1:
                    transpose(b, nc.scalar)  # after exps: block at slot end
                if b > 0:
                    ctx(b - 1)
                    flush(b - 1)
                tree(b)
            beta(BPC - 1)
            ctx(BPC - 1)
            flush(BPC - 1)
    return nc


def _get_nc():
    if "nc" not in _CACHE:
        nc = _build_nc()
        nc.finalize()
        _CACHE["nc"] = nc
    return _CACHE["nc"]


def _to_bf16(x_np):
    """Round-to-nearest fp32 -> bf16 via bit twiddling."""
    v = x_np.view(np.uint32)
    r = ((v + 0x7FFF + ((v >> 16) & 1)) >> 16).astype(np.uint16)
    return r


def _run_device(x_np, trace=False, tmpdir=None):
    """x_np: [64, 256, 64, 64] fp32 -> list of 8 per-core result dicts."""
    import ml_dtypes
    from concourse.bass_utils import run_bass_kernel_spmd

    nc = _get_nc()
    xb = _to_bf16(np.ascontiguousarray(x_np).reshape(-1)).view(
        ml_dtypes.bfloat16).reshape(NCORES, BPC, 2, 128, S)
    # device layout: [BPC, 128, 2S] with channel half c+128 at free offset S
    xs = np.ascontiguousarray(np.transpose(xb, (0, 1, 3, 2, 4))).reshape(
        NCORES, BPC, 128, 2 * S)
    wm = _CACHE["w_mask"].reshape(C)
    wmb0 = np.ascontiguousarray(
        np.repeat(_to_bf16(wm[0:128].astype(np.float32))[:, None], 128,
                  axis=1)).view(ml_dtypes.bfloat16)
    wmb1 = np.ascontiguousarray(
        np.repeat(_to_bf16(wm[128:256].astype(np.float32))[:, None], 128,
                  axis=1)).view(ml_dtypes.bfloat16)
    in_maps = [
        {"x": xs[i], "wmb0": wmb0, "wmb1": wmb1}
        for i in range(NCORES)
    ]
    res = run_bass_kernel_spmd(nc, in_maps, core_ids=list(range(NCORES)),
                               trace=trace, tmpdir=tmpdir)
    return res


def kernel(x, w_mask, b_mask, w_cm1, b_cm1, ln_w, ln_b, w_cm2, b_cm2,
           w_net1, w_net2, w_fc, bn_w, bn_b, bn_mean, bn_var, w_kfc):
    x = np.asarray(x, dtype=np.float32)
    _CACHE["w_mask"] = np.asarray(w_mask, dtype=np.float32)
    res = _run_device(x)

    # ---- gather device results
    beta_sums = np.zeros([B, C], np.float32)
    ctx_sums = np.zeros([B, C], np.float32)
    zs = np.zeros([B], np.float32)
    cmax_sums = np.zeros([B], np.float32)
    for i in range(NCORES):
        o = np.asarray(res.results[i]["out"], np.float32)  # [BPC, 128, 8]
        for bb in range(BPC):
            g = i * BPC + bb
            beta_sums[g, 0:128] = o[bb, :, 0]
            beta_sums[g, 128:256] = o[bb, :, 1]
            ctx_sums[g, 0:128] = o[bb, :, 2]
            ctx_sums[g, 128:256] = o[bb, :, 3]
            o2 = np.asarray(res.results[i]["out2"][bb], np.float32)
            cmax_sums[g] = o2.reshape(128, 32, 32).max(-1).sum()
            zs[g] = o[bb, 0, 5] + o[bb, 0, 6]

    # ---- tiny epilogue head on host (mirrors reference.py)
    w_cm1 = np.asarray(w_cm1, np.float32); b_cm1 = np.asarray(b_cm1, np.float32)
    ln_w = np.asarray(ln_w, np.float32); ln_b = np.asarray(ln_b, np.float32)
    w_cm2 = np.asarray(w_cm2, np.float32); b_cm2 = np.asarray(b_cm2, np.float32)
    w_net1 = np.asarray(w_net1, np.float32); w_net2 = np.asarray(w_net2, np.float32)
    w_fc = np.asarray(w_fc, np.float32); bn_w = np.asarray(bn_w, np.float32)
    bn_b = np.asarray(bn_b, np.float32); bn_mean = np.asarray(bn_mean, np.float32)
    bn_var = np.asarray(bn_var, np.float32); w_kfc = np.asarray(w_kfc, np.float32)

    from scipy.special import erf  # exact gelu, matches jax approximate=False

    beta_c = beta_sums / S
    context = ctx_sums / zs[:, None]
    a = beta_sums.sum(axis=1) / (C * S)
    mm = cmax_sums / S
    beta_s = np.zeros([B, C], np.float32)
    beta_s[:, 0::2] = a[:, None]
    beta_s[:, 1::2] = mm[:, None]

    t = context @ w_cm1.T + b_cm1
    mu = t.mean(axis=-1, keepdims=True)
    var = ((t - mu) ** 2).mean(axis=-1, keepdims=True)
    t = (t - mu) / np.sqrt(var + EPS) * ln_w + ln_b
    t = t * 0.5 * (1.0 + erf(t / np.sqrt(2.0)))
    beta_g = t @ w_cm2.T + b_cm2

    out = beta_c + beta_g + beta_s
    out = np.maximum(out @ w_net1.T, 0.0) @ w_net2.T  # [B, K]

    ka = out @ w_fc.T
    ka = (ka - bn_mean) / np.sqrt(bn_var + EPS) * bn_w + bn_b
    kat = 1.0 / (1.0 + np.exp(-(np.maximum(ka, 0.0) @ w_kfc.T)))
    out = out * kat
    out = out / TEMP
    out = out - out.max(axis=-1, keepdims=True)
    e = np.exp(out)
    return (e / e.sum(axis=-1, keepdims=True)).astype(np.float32)


# revision 30
# speedup vs baseline: 1.0357x; 1.0357x over previous
"""Trainium2 Bass kernel for nn_Attention_63273458205325.

Data-parallel over batch: 64 images -> 8 NeuronCores x 8 images.
Host casts x to bf16 (halves HBM traffic); device computes, per image,
the four memory-bound global reductions over x[b] (256x4096):
  - beta row-sums  (per-channel sum over spatial)       [256]
  - mask logits m = w_mask . x  -> exp -> Z and the
    softmax-weighted context sums  sum_s x[c,s]*e[s]    [256]
  - mean over spatial of (max over channels)            scalar
Engine split (per image):
  PE : mask matmul with w_mask replicated into all 128 stationary
       columns -> m arrives pre-broadcast [128, 2048] in PSUM.
  ACT: exp(PSUM)->bf16 SBUF e_b (Z rides accum), plus the two
       beta copy-accum passes.
  DVE: fused multiply-accumulate for ctx (scalar_tensor_tensor),
       channel fold (max), and the post-transpose max tree.
  DMA: x loads on the sync ring; SBUF->SBUF transposes (hardware
       XBAR) split across the sync+scalar rings.
The per-engine queues are software-pipelined one image deep: image
b's slot runs ctx/beta of image b-1 so the in-order DVE/ACT queues
never head-of-line block on the exp->e_b chain. DMA-transpose blocks
its issuing engine until the transfer lands, so even images go on the
(otherwise idle) sync ring and odd images issue from scalar after the
exps, landing the block in dead time.
The tiny [B,8] epilogue head runs on host.
"""

import sys

import numpy as np

sys.path.insert(0, "/opt/trn_rl_repo")

B, C, H, W = 64, 256, 64, 64
S = H * W  # 4096
NCORES = 8
BPC = B // NCORES  # images per core
RATIO, K = 16, 8
PLANES = C // 2
HIDDEN = C // RATIO
TEMP = 30.0
EPS = 1e-5

_CACHE = {}


def _build_nc():
    import concourse.bacc as bacc
    import concourse.mybir as mybir
    from concourse.tile import TileContext

    f32 = mybir.dt.float32
    bf16 = mybir.dt.bfloat16
    AF = mybir.ActivationFunctionType
    ALU = mybir.AluOpType

    nc = bacc.Bacc(None, target_bir_lowering=False)

    x_ext = nc.declare_dram_parameter("x", [BPC, 128, 2 * S], bf16,
                                      isOutput=False)
    wmb0_ext = nc.declare_dram_parameter("wmb0", [128, 128], bf16,
                                         isOutput=False)
    wmb1_ext = nc.declare_dram_parameter("wmb1", [128, 128], bf16,
                                         isOutput=False)
    out_ext = nc.declare_dram_parameter("out", [BPC, 128, 8], f32,
                                        isOutput=True)
    out2_ext = nc.declare_dram_parameter("out2", [BPC, 128, 256], bf16,
                                         isOutput=True)

    with TileContext(nc) as tc:
        with (
            tc.tile_pool(name="const", bufs=1) as cpool,
            tc.tile_pool(name="xin", bufs=3) as xpool,
            tc.tile_pool(name="work", bufs=3) as wpool,
            tc.tile_pool(name="small", bufs=3) as spool,
            tc.tile_pool(name="stg", bufs=3) as gpool,
            tc.tile_pool(name="psum", bufs=2, space="PSUM") as ppool,
        ):
            scr = cpool.tile([128, S], bf16)
            dummy = cpool.tile([128, 1], bf16)
            wmb0 = cpool.tile([128, 128], bf16)
            wmb1 = cpool.tile([128, 128], bf16)

            xbs, ebs, stages = {}, {}, {}

            def load(b):
                xb = xpool.tile([128, 2 * S], bf16, tag="xb")
                nc.sync.dma_start(out=xb[:], in_=x_ext[b])
                xbs[b] = xb
                stage = gpool.tile([128, 8], f32, tag="stage")
                nc.gpsimd.memset(stage[:], 0.0)
                stages[b] = stage

            pms = {}

            def fold(b):
                xb = xbs[b]
                pm = wpool.tile([128, S], bf16, tag="pm")
                nc.vector.tensor_tensor(out=pm[:], in0=xb[:, 0:S],
                                        in1=xb[:, S:2 * S], op=ALU.max)
                pms[b] = pm

            def transpose(b, eng):
                pm = pms.pop(b)
                pmT = wpool.tile([128, 32, 128], bf16, tag="pmT")
                eng.dma_start(out=pmT[:], in_=pm[:], transpose=True)
                pms[(b, "T")] = pmT

            def tree(b):
                pmT = pms.pop((b, "T"))
                t1 = spool.tile([128, 32, 64], bf16, tag="t1")
                nc.vector.tensor_tensor(out=t1[:], in0=pmT[:, :, 0:64],
                                        in1=pmT[:, :, 64:128], op=ALU.max)
                t2 = spool.tile([128, 32, 32], bf16, tag="t2")
                nc.vector.tensor_tensor(out=t2[:], in0=t1[:, :, 0:32],
                                        in1=t1[:, :, 32:64], op=ALU.max)
                t3 = spool.tile([128, 32, 16], bf16, tag="t3")
                nc.vector.tensor_tensor(out=t3[:], in0=t2[:, :, 0:16],
                                        in1=t2[:, :, 16:32], op=ALU.max)
                t4 = spool.tile([128, 32, 8], bf16, tag="t4")
                nc.vector.tensor_tensor(out=t4[:], in0=t3[:, :, 0:8],
                                        in1=t3[:, :, 8:16], op=ALU.max)
                # host finishes the last 3 max levels + the spatial sum
                nc.sync.dma_start(out=out2_ext[b],
                                  in_=t4[:].rearrange("p a b -> p (a b)"))

            def mask_exp(b):
                """PE mask-broadcast matmuls + ACT exp -> e_b, Z."""
                xb, stage = xbs[b], stages[b]
                e_b = wpool.tile([128, S], bf16, tag="eb")
                for g in range(2):
                    mb = ppool.tile([128, 2048], f32, tag="mb")
                    for j in range(4):
                        sl = slice(2048 * g + 512 * j,
                                   2048 * g + 512 * (j + 1))
                        nc.tensor.matmul(mb[:, 512 * j:512 * (j + 1)],
                                         lhsT=wmb0[:], rhs=xb[:, sl],
                                         start=True, stop=False)
                    for j in range(4):
                        sl = slice(S + 2048 * g + 512 * j,
                                   S + 2048 * g + 512 * (j + 1))
                        nc.tensor.matmul(mb[:, 512 * j:512 * (j + 1)],
                                         lhsT=wmb1[:], rhs=xb[:, sl],
                                         start=False, stop=True)
                    nc.scalar.activation(e_b[:, 2048 * g:2048 * (g + 1)],
                                         mb[:], AF.Exp,
                                         accum_out=stage[:, 5 + g:6 + g])
                ebs[b] = e_b

            def beta(b):
                """beta row sums on ACT (copy with accumulator)."""
                xb, stage = xbs[b], stages[b]
                nc.scalar.activation(scr[:], xb[:, 0:S], AF.Copy,
                                     accum_out=stage[:, 0:1])
                nc.scalar.activation(scr[:], xb[:, S:2 * S], AF.Copy,
                                     accum_out=stage[:, 1:2])

            def ctx(b):
                """ctx fused multiply-accumulate on DVE."""
                xb, stage, e_b = xbs[b], stages[b], ebs[b]
                nc.vector.scalar_tensor_tensor(
                    out=dummy[:].broadcast_to([128, S]),
                    in0=xb[:, 0:S], scalar=1.0, in1=e_b[:],
                    op0=ALU.mult, op1=ALU.mult, accum_out=stage[:, 2:3])
                nc.vector.scalar_tensor_tensor(
                    out=dummy[:].broadcast_to([128, S]),
                    in0=xb[:, S:2 * S], scalar=1.0, in1=e_b[:],
                    op0=ALU.mult, op1=ALU.mult, accum_out=stage[:, 3:4])

            def flush(b):
                nc.sync.dma_start(out=out_ext[b], in_=stages[b])
                del xbs[b], ebs[b], stages[b]

            # software pipeline, one image deep
            load(0)
            nc.sync.dma_start(out=wmb0[:], in_=wmb0_ext[:, :])
            nc.sync.dma_start(out=wmb1[:], in_=wmb1_ext[:, :])
            for b in range(BPC):
                if b + 1 < BPC:
                    load(b + 1)
                fold(b)
                if b % 2 == 0:
                    transpose(b, nc.sync)   # blocks the idle sync engine
                if b == BPC - 1:
                    mask_exp(b)     # last slot: unblock the drain's ctx
                if b > 0:
                    beta(b - 1)     # ACT filler while PE streams matmuls
                if b < BPC - 1:
                    mask_exp(b)
                if b % 2 == 1:
                    transpose(b, nc.scalar)  # after exps: block at slot end
                if b > 0:
                    ctx(b - 1)
                    flush(b - 1)
                tree(b)
            beta(BPC - 1)
            ctx(BPC - 1)
            flush(BPC - 1)
    return nc


def _get_nc():
    if "nc" not in _CACHE:
        nc = _build_nc()
        nc.finalize()
        _CACHE["nc"] = nc
    return _CACHE["nc"]


def _to_bf16(x_np):
    """Round-to-nearest fp32 -> bf16 via bit twiddling."""
    v = x_np.view(np.uint32)
    r = ((v + 0x7FFF + ((v >> 16) & 1)) >> 16).astype(np.uint16)
    return r


def _run_device(x_np, trace=False, tmpdir=None):
    """x_np: [64, 256, 64, 64] fp32 -> list of 8 per-core result dicts."""
    import ml_dtypes
    from concourse.bass_utils import run_bass_kernel_spmd

    nc = _get_nc()
    xb = _to_bf16(np.ascontiguousarray(x_np).reshape(-1)).view(
        ml_dtypes.bfloat16).reshape(NCORES, BPC, 2, 128, S)
    # device layout: [BPC, 128, 2S] with channel half c+128 at free offset S
    xs = np.ascontiguousarray(np.transpose(xb, (0, 1, 3, 2, 4))).reshape(
        NCORES, BPC, 128, 2 * S)
    wm = _CACHE["w_mask"].reshape(C)
    wmb0 = np.ascontiguousarray(
        np.repeat(_to_bf16(wm[0:128].astype(np.float32))[:, None], 128,
                  axis=1)).view(ml_dtypes.bfloat16)
    wmb1 = np.ascontiguousarray(
        np.repeat(_to_bf16(wm[128:256].astype(np.float32))[:, None], 128,
                  axis=1)).view(ml_dtypes.bfloat16)
    in_maps = [
        {"x": xs[i], "wmb0": wmb0, "wmb1": wmb1}
        for i in range(NCORES)
    ]
    res = run_bass_kernel_spmd(nc, in_maps, core_ids=list(range(NCORES)),
                               trace=trace, tmpdir=tmpdir)
    return res


def kernel(x, w_mask, b_mask, w_cm1, b_cm1, ln_w, ln_b, w_cm2, b_cm2,
           w_net1, w_net2, w_fc, bn_w, bn_b, bn_mean, bn_var, w_kfc):
    x = np.asarray(x, dtype=np.float32)
    _CACHE["w_mask"] = np.asarray(w_mask, dtype=np.float32)
    res = _run_device(x)

    # ---- gather device results
    beta_sums = np.zeros([B, C], np.float32)
    ctx_sums = np.zeros([B, C], np.float32)
    zs = np.zeros([B], np.float32)
    cmax_sums = np.zeros([B], np.float32)
    for i in range(NCORES):
        o = np.asarray(res.results[i]["out"], np.float32)  # [BPC, 128, 8]
        for bb in range(BPC):
            g = i * BPC + bb
            beta_sums[g, 0:128] = o[bb, :, 0]
            beta_sums[g, 128:256] = o[bb, :, 1]
            ctx_sums[g, 0:128] = o[bb, :, 2]
            ctx_sums[g, 128:256] = o[bb, :, 3]
            o2 = np.asarray(res.results[i]["out2"][bb], np.float32)
            cmax_sums[g] = o2.reshape(128, 32, 8).max(-1).sum()
            zs[g] = o[bb, 0, 5] + o[bb, 0, 6]

    # ---- tiny epilogue head on host (mirrors reference.py)
    w_cm1 = np.asarray(w_cm1, np.float32); b_cm1 = np.asarray(b_cm1, np.float32)
    ln_w = np.asarray(ln_w, np.float32); ln_b = np.asarray(ln_b, np.float32)
    w_cm2 = np.asarray(w_cm2, np.float32); b_cm2 = np.asarray(b_cm2, np.float32)
    w_net1 = np.asarray(w_net1, np.float32); w_net2 = np.asarray(w_net2, np.float32)
    w_fc = np.asarray(w_fc, np.float32); bn_w = np.asarray(bn_w, np.float32)
    bn_b = np.asarray(bn_b, np.float32); bn_mean = np.asarray(bn_mean, np.float32)
    bn_var = np.asarray(bn_var, np.float32); w_kfc = np.asarray(w_kfc, np.float32)

    from scipy.special import erf  # exact gelu, matches jax approximate=False

    beta_c = beta_sums / S
    context = ctx_sums / zs[:, None]
    a = beta_sums.sum(axis=1) / (C * S)
    mm = cmax_sums / S
    beta_s = np.zeros([B, C], np.float32)
    beta_s[:, 0::2] = a[:, None]
    beta_s[:, 1::2] = mm[:, None]

    t = context @ w_cm1.T + b_cm1
    mu = t.mean(axis=-1, keepdims=True)
    var = ((t - mu) ** 2).mean(axis=-1, keepdims=True)
    t = (t - mu) / np.sqrt(var + EPS) * ln_w + ln_b
    t = t * 0.5 * (1.0 + erf(t / np.sqrt(2.0)))
    beta_g = t @ w_cm2.T + b_cm2

    out = beta_c + beta_g + beta_s
    out = np.maximum(out @ w_net1.T, 0.0) @ w_net2.T  # [B, K]

    ka = out @ w_fc.T
    ka = (ka - bn_mean) / np.sqrt(bn_var + EPS) * bn_w + bn_b
    kat = 1.0 / (1.0 + np.exp(-(np.maximum(ka, 0.0) @ w_kfc.T)))
    out = out * kat
    out = out / TEMP
    out = out - out.max(axis=-1, keepdims=True)
    e = np.exp(out)
    return (e / e.sum(axis=-1, keepdims=True)).astype(np.float32)


# revision 31
# speedup vs baseline: 1.0677x; 1.0309x over previous
"""Trainium2 Bass kernel for nn_Attention_63273458205325.

Data-parallel over batch: 64 images -> 8 NeuronCores x 8 images.
Host casts x to bf16 (halves HBM traffic); device computes, per image,
the four memory-bound global reductions over x[b] (256x4096):
  - beta row-sums  (per-channel sum over spatial)       [256]
  - mask logits m = w_mask . x  -> exp -> Z and the
    softmax-weighted context sums  sum_s x[c,s]*e[s]    [256]
  - mean over spatial of (max over channels)            scalar
Engine split (per image):
  PE : mask matmul with w_mask replicated into all 128 stationary
       columns -> m arrives pre-broadcast [128, 2048] in PSUM.
  ACT: exp(PSUM)->bf16 SBUF e_b (Z rides accum), plus the two
       beta copy-accum passes.
  DVE: fused multiply-accumulate for ctx (scalar_tensor_tensor),
       channel fold (max), and the first 4 levels of the post-
       transpose max tree; the last 3 levels + spatial sum finish on
       host from the shipped [128,256] tree midpoint (64KB/image).
  DMA: x loads on the sync ring; SBUF->SBUF transposes (hardware
       XBAR) split across the sync+scalar rings.
The per-engine queues are software-pipelined one image deep: image
b's slot runs ctx/beta of image b-1 so the in-order DVE/ACT queues
never head-of-line block on the exp->e_b chain. DMA-transpose blocks
its issuing engine until the transfer lands, so even images go on the
(otherwise idle) sync ring and odd images issue from scalar after the
exps, landing the block in dead time.
The tiny [B,8] epilogue head runs on host.
"""

import sys

import numpy as np

sys.path.insert(0, "/opt/trn_rl_repo")

B, C, H, W = 64, 256, 64, 64
S = H * W  # 4096
NCORES = 8
BPC = B // NCORES  # images per core
RATIO, K = 16, 8
PLANES = C // 2
HIDDEN = C // RATIO
TEMP = 30.0
EPS = 1e-5

_CACHE = {}


def _build_nc():
    import concourse.bacc as bacc
    import concourse.mybir as mybir
    from concourse.tile import TileContext

    f32 = mybir.dt.float32
    bf16 = mybir.dt.bfloat16
    AF = mybir.ActivationFunctionType
    ALU = mybir.AluOpType

    nc = bacc.Bacc(None, target_bir_lowering=False)

    x_ext = nc.declare_dram_parameter("x", [BPC, 128, 2 * S], bf16,
                                      isOutput=False)
    wmb0_ext = nc.declare_dram_parameter("wmb0", [128, 128], bf16,
                                         isOutput=False)
    wmb1_ext = nc.declare_dram_parameter("wmb1", [128, 128], bf16,
                                         isOutput=False)
    out_ext = nc.declare_dram_parameter("out", [BPC, 128, 8], f32,
                                        isOutput=True)
    out2_ext = nc.declare_dram_parameter("out2", [BPC, 128, 256], bf16,
                                         isOutput=True)

    with TileContext(nc) as tc:
        with (
            tc.tile_pool(name="const", bufs=1) as cpool,
            tc.tile_pool(name="xin", bufs=3) as xpool,
            tc.tile_pool(name="work", bufs=3) as wpool,
            tc.tile_pool(name="small", bufs=3) as spool,
            tc.tile_pool(name="stg", bufs=3) as gpool,
            tc.tile_pool(name="psum", bufs=2, space="PSUM") as ppool,
        ):
            scr = cpool.tile([128, S], bf16)
            dummy = cpool.tile([128, 1], bf16)
            wmb0 = cpool.tile([128, 128], bf16)
            wmb1 = cpool.tile([128, 128], bf16)

            xbs, ebs, stages = {}, {}, {}

            def load(b):
                xb = xpool.tile([128, 2 * S], bf16, tag="xb")
                nc.sync.dma_start(out=xb[:], in_=x_ext[b])
                xbs[b] = xb
                stage = gpool.tile([128, 8], f32, tag="stage")
                nc.gpsimd.memset(stage[:], 0.0)
                stages[b] = stage

            pms = {}

            def fold(b):
                xb = xbs[b]
                pm = wpool.tile([128, S], bf16, tag="pm")
                nc.vector.tensor_tensor(out=pm[:], in0=xb[:, 0:S],
                                        in1=xb[:, S:2 * S], op=ALU.max)
                pms[b] = pm

            def transpose(b, eng):
                pm = pms.pop(b)
                pmT = wpool.tile([128, 32, 128], bf16, tag="pmT")
                eng.dma_start(out=pmT[:], in_=pm[:], transpose=True)
                pms[(b, "T")] = pmT

            def tree(b):
                pmT = pms.pop((b, "T"))
                t1 = spool.tile([128, 32, 64], bf16, tag="t1")
                nc.vector.tensor_tensor(out=t1[:], in0=pmT[:, :, 0:64],
                                        in1=pmT[:, :, 64:128], op=ALU.max)
                t2 = spool.tile([128, 32, 32], bf16, tag="t2")
                nc.vector.tensor_tensor(out=t2[:], in0=t1[:, :, 0:32],
                                        in1=t1[:, :, 32:64], op=ALU.max)
                t3 = spool.tile([128, 32, 16], bf16, tag="t3")
                nc.vector.tensor_tensor(out=t3[:], in0=t2[:, :, 0:16],
                                        in1=t2[:, :, 16:32], op=ALU.max)
                t4 = spool.tile([128, 32, 8], bf16, tag="t4")
                nc.vector.tensor_tensor(out=t4[:], in0=t3[:, :, 0:8],
                                        in1=t3[:, :, 8:16], op=ALU.max)
                # host finishes the last 3 max levels + the spatial sum
                nc.sync.dma_start(out=out2_ext[b],
                                  in_=t4[:].rearrange("p a b -> p (a b)"))

            def mask_exp(b):
                """PE mask-broadcast matmuls + ACT exp -> e_b, Z."""
                xb, stage = xbs[b], stages[b]
                e_b = wpool.tile([128, S], bf16, tag="eb")
                for g in range(2):
                    mb = ppool.tile([128, 2048], f32, tag="mb")
                    for j in range(4):
                        sl = slice(2048 * g + 512 * j,
                                   2048 * g + 512 * (j + 1))
                        nc.tensor.matmul(mb[:, 512 * j:512 * (j + 1)],
                                         lhsT=wmb0[:], rhs=xb[:, sl],
                                         start=True, stop=False)
                    for j in range(4):
                        sl = slice(S + 2048 * g + 512 * j,
                                   S + 2048 * g + 512 * (j + 1))
                        nc.tensor.matmul(mb[:, 512 * j:512 * (j + 1)],
                                         lhsT=wmb1[:], rhs=xb[:, sl],
                                         start=False, stop=True)
                    nc.scalar.activation(e_b[:, 2048 * g:2048 * (g + 1)],
                                         mb[:], AF.Exp,
                                         accum_out=stage[:, 5 + g:6 + g])
                ebs[b] = e_b

            def beta(b):
                """beta row sums on ACT (copy with accumulator)."""
                xb, stage = xbs[b], stages[b]
                nc.scalar.activation(scr[:], xb[:, 0:S], AF.Copy,
                                     accum_out=stage[:, 0:1])
                nc.scalar.activation(scr[:], xb[:, S:2 * S], AF.Copy,
                                     accum_out=stage[:, 1:2])

            def ctx(b):
                """ctx fused multiply-accumulate on DVE."""
                xb, stage, e_b = xbs[b], stages[b], ebs[b]
                nc.vector.scalar_tensor_tensor(
                    out=dummy[:].broadcast_to([128, S]),
                    in0=xb[:, 0:S], scalar=1.0, in1=e_b[:],
                    op0=ALU.mult, op1=ALU.mult, accum_out=stage[:, 2:3])
                nc.vector.scalar_tensor_tensor(
                    out=dummy[:].broadcast_to([128, S]),
                    in0=xb[:, S:2 * S], scalar=1.0, in1=e_b[:],
                    op0=ALU.mult, op1=ALU.mult, accum_out=stage[:, 3:4])

            def flush(b):
                nc.sync.dma_start(out=out_ext[b], in_=stages[b])
                del xbs[b], ebs[b], stages[b]

            # software pipeline, one image deep
            load(0)
            nc.sync.dma_start(out=wmb0[:], in_=wmb0_ext[:, :])
            nc.sync.dma_start(out=wmb1[:], in_=wmb1_ext[:, :])
            for b in range(BPC):
                if b + 1 < BPC:
                    load(b + 1)
                fold(b)
                if b % 2 == 0:
                    transpose(b, nc.sync)   # blocks the idle sync engine
                if b == BPC - 1:
                    mask_exp(b)     # last slot: unblock the drain's ctx
                if b > 0:
                    beta(b - 1)     # ACT filler while PE streams matmuls
                if b < BPC - 1:
                    mask_exp(b)
                if b % 2 == 1:
                    transpose(b, nc.scalar)  # after exps: block at slot end
                if b > 0:
                    ctx(b - 1)
                    flush(b - 1)
                tree(b)
            beta(BPC - 1)
            ctx(BPC - 1)
            flush(BPC - 1)
    return nc


def _get_nc():
    if "nc" not in _CACHE:
        nc = _build_nc()
        nc.finalize()
        _CACHE["nc"] = nc
    return _CACHE["nc"]


def _to_bf16(x_np):
    """Round-to-nearest fp32 -> bf16 via bit twiddling."""
    v = x_np.view(np.uint32)
    r = ((v + 0x7FFF + ((v >> 16) & 1)) >> 16).astype(np.uint16)
    return r


def _run_device(x_np, trace=False, tmpdir=None):
    """x_np: [64, 256, 64, 64] fp32 -> list of 8 per-core result dicts."""
    import ml_dtypes
    from concourse.bass_utils import run_bass_kernel_spmd

    nc = _get_nc()
    xb = _to_bf16(np.ascontiguousarray(x_np).reshape(-1)).view(
        ml_dtypes.bfloat16).reshape(NCORES, BPC, 2, 128, S)
    # device layout: [BPC, 128, 2S] with channel half c+128 at free offset S
    xs = np.ascontiguousarray(np.transpose(xb, (0, 1, 3, 2, 4))).reshape(
        NCORES, BPC, 128, 2 * S)
    wm = _CACHE["w_mask"].reshape(C)
    wmb0 = np.ascontiguousarray(
        np.repeat(_to_bf16(wm[0:128].astype(np.float32))[:, None], 128,
                  axis=1)).view(ml_dtypes.bfloat16)
    wmb1 = np.ascontiguousarray(
        np.repeat(_to_bf16(wm[128:256].astype(np.float32))[:, None], 128,
                  axis=1)).view(ml_dtypes.bfloat16)
    in_maps = [
        {"x": xs[i], "wmb0": wmb0, "wmb1": wmb1}
        for i in range(NCORES)
    ]
    res = run_bass_kernel_spmd(nc, in_maps, core_ids=list(range(NCORES)),
                               trace=trace, tmpdir=tmpdir)
    return res


def kernel(x, w_mask, b_mask, w_cm1, b_cm1, ln_w, ln_b, w_cm2, b_cm2,
           w_net1, w_net2, w_fc, bn_w, bn_b, bn_mean, bn_var, w_kfc):
    x = np.asarray(x, dtype=np.float32)
    _CACHE["w_mask"] = np.asarray(w_mask, dtype=np.float32)
    res = _run_device(x)

    # ---- gather device results
    beta_sums = np.zeros([B, C], np.float32)
    ctx_sums = np.zeros([B, C], np.float32)
    zs = np.zeros([B], np.float32)
    cmax_sums = np.zeros([B], np.float32)
    for i in range(NCORES):
        o = np.asarray(res.results[i]["out"], np.float32)  # [BPC, 128, 8]
        for bb in range(BPC):
            g = i * BPC + bb
            beta_sums[g, 0:128] = o[bb, :, 0]
            beta_sums[g, 128:256] = o[bb, :, 1]
            ctx_sums[g, 0:128] = o[bb, :, 2]
            ctx_sums[g, 128:256] = o[bb, :, 3]
            o2 = np.asarray(res.results[i]["out2"][bb], np.float32)
            cmax_sums[g] = o2.reshape(128, 32, 8).max(-1).sum()
            zs[g] = o[bb, 0, 5] + o[bb, 0, 6]

    # ---- tiny epilogue head on host (mirrors reference.py)
    w_cm1 = np.asarray(w_cm1, np.float32); b_cm1 = np.asarray(b_cm1, np.float32)
    ln_w = np.asarray(ln_w, np.float32); ln_b = np.asarray(ln_b, np.float32)
    w_cm2 = np.asarray(w_cm2, np.float32); b_cm2 = np.asarray(b_cm2, np.float32)
    w_net1 = np.asarray(w_net1, np.float32); w_net2 = np.asarray(w_net2, np.float32)
    w_fc = np.asarray(w_fc, np.float32); bn_w = np.asarray(bn_w, np.float32)
    bn_b = np.asarray(bn_b, np.float32); bn_mean = np.asarray(bn_mean, np.float32)
    bn_var = np.asarray(bn_var, np.float32); w_kfc = np.asarray(w_kfc, np.float32)

    from scipy.special import erf  # exact gelu, matches jax approximate=False

    beta_c = beta_sums / S
    context = ctx_sums / zs[:, None]
    a = beta_sums.sum(axis=1) / (C * S)
    mm = cmax_sums / S
    beta_s = np.zeros([B, C], np.float32)
    beta_s[:, 0::2] = a[:, None]
    beta_s[:, 1::2] = mm[:, None]

    t = context @ w_cm1.T + b_cm1
    mu = t.mean(axis=-1, keepdims=True)
    var = ((t - mu) ** 2).mean(axis=-1, keepdims=True)
    t = (t - mu) / np.sqrt(var + EPS) * ln_w + ln_b
    t = t * 0.5 * (1.0 + erf(t / np.sqrt(2.0)))
    beta_g = t @ w_cm2.T + b_cm2

    out = beta_c + beta_g + beta_s
    out = np.maximum(out @ w_net1.T, 0.0) @ w_net2.T  # [B, K]

    ka = out @ w_fc.T
    ka = (ka - bn_mean) / np.sqrt(bn_var + EPS) * bn_w + bn_b
    kat = 1.0 / (1.0 + np.exp(-(np.maximum(ka, 0.0) @ w_kfc.T)))
    out = out * kat
    out = out / TEMP
    out = out - out.max(axis=-1, keepdims=True)
    e = np.exp(out)
    return (e / e.sum(axis=-1, keepdims=True)).astype(np.float32)
